# revision 8
# baseline (speedup 1.0000x reference)
"""HalfKP NNUE forward pass on 8 Trainium2 NeuronCores.

Strategy (data-parallel over batch, per sharding hint):
  - Each core handles 1024 of the 8192 batch rows.
  - Host prep: transpose ft_W -> [40960, 256] row-major; per core, compact
    the table to the rows that core actually touches (~28.6K unique < 2^15,
    so indices fit the int16 requirement of the gpsimd dma_gather
    instruction). Row 32767 of each per-core table is all-zero; pad indices
    (-1) map there so the gather needs no masking.
  - Device: per 128-row tile, one dma_gather pulls 8192 table rows (1KB
    each) into SBUF [128, 64, 256] laid out [row, slot, :]; DVE tree-reduces
    white slots (0:32) and black slots (32:64); bias + clip(0,127); stm mix;
    tiny MLP via PE matmuls; sigmoid on ACT.
"""

import sys

sys.path.insert(0, "/opt/trn_rl_repo")

import numpy as np

import concourse.bass as bass
import concourse.tile as tile
from concourse import bacc, mybir
from concourse.masks import make_identity

INPUT_SIZE = 40960
HIDDEN = 256
MAX_FEATS = 32
BATCH = 8192
N_CORES = 8
ROWS = BATCH // N_CORES  # 1024 rows per core
SLOTS = 2 * MAX_FEATS  # 64 gather slots per row (white + black)
P = 128
N_TILES = ROWS // P  # 8
TABLE_ROWS = 32768  # per-core compacted table, int16-indexable
ZERO_ROW = TABLE_ROWS - 1  # all-zero row for -1 padding
IDX_COLS = (P * SLOTS) // 16  # 512: dma_gather wraps indices in 16 partitions

dt = mybir.dt


def build_nc():
    nc = bacc.Bacc(
        "TRN2",
        target_bir_lowering=False,
        debug=False,
        num_devices=N_CORES,
    )

    idx_d = nc.dram_tensor(
        "idx", [N_TILES, P, IDX_COLS], dt.int16, kind="ExternalInput"
    )
    table_d = nc.dram_tensor(
        "table", [TABLE_ROWS, HIDDEN], dt.float32, kind="ExternalInput"
    )
    stm_d = nc.dram_tensor("stm", [ROWS, 1], dt.float32, kind="ExternalInput")
    ftb_d = nc.dram_tensor("ftb", [1, HIDDEN], dt.float32, kind="ExternalInput")
    w1t_d = nc.dram_tensor("w1t", [2 * HIDDEN, 32], dt.float32, kind="ExternalInput")
    b1_d = nc.dram_tensor("b1", [32, 1], dt.float32, kind="ExternalInput")
    w2t_d = nc.dram_tensor("w2t", [32, 32], dt.float32, kind="ExternalInput")
    b2_d = nc.dram_tensor("b2", [32, 1], dt.float32, kind="ExternalInput")
    wot_d = nc.dram_tensor("wot", [32, 1], dt.float32, kind="ExternalInput")
    bo_d = nc.dram_tensor("bo", [1, 1], dt.float32, kind="ExternalInput")
    out_d = nc.dram_tensor("out", [1, ROWS], dt.float32, kind="ExternalOutput")

    with tile.TileContext(nc) as tc:
        with (
            tc.tile_pool(name="const", bufs=1) as constp,
            tc.tile_pool(name="gath", bufs=2) as gpool,
            tc.tile_pool(name="work", bufs=2) as wpool,
            tc.tile_pool(name="psum", bufs=2, space="PSUM") as ppool,
            tc.tile_pool(name="outp", bufs=1) as outp,
        ):
            ident = constp.tile([P, P], dt.float32)
            make_identity(nc, ident[:])
            bias_sb = constp.tile([P, HIDDEN], dt.float32)
            nc.sync.dma_start(bias_sb[:], ftb_d[:].to_broadcast((P, HIDDEN)))
            w1t_sb = constp.tile([P, 4, 32], dt.float32)
            for c in range(4):
                nc.sync.dma_start(w1t_sb[:, c, :], w1t_d[c * P : (c + 1) * P, :])
            w2t_sb = constp.tile([32, 32], dt.float32)
            nc.sync.dma_start(w2t_sb[:], w2t_d[:, :])
            b1_sb = constp.tile([32, 1], dt.float32)
            nc.sync.dma_start(b1_sb[:], b1_d[:, :])
            b2_sb = constp.tile([32, 1], dt.float32)
            nc.sync.dma_start(b2_sb[:], b2_d[:, :])
            wot_sb = constp.tile([32, 1], dt.float32)
            nc.sync.dma_start(wot_sb[:], wot_d[:, :])
            bo_sb = constp.tile([1, 1], dt.float32)
            nc.sync.dma_start(bo_sb[:], bo_d[:, :])

            out_sb = outp.tile([1, ROWS], dt.float32)

            for t in range(N_TILES):
                r0 = t * P
                idx_sb = wpool.tile([P, IDX_COLS], dt.int16, tag="idx")
                nc.sync.dma_start(idx_sb[:], idx_d[t, :, :])
                stm_sb = wpool.tile([P, 1], dt.float32, tag="stm")
                nc.sync.dma_start(stm_sb[:], stm_d[r0 : r0 + P, :])

                G = gpool.tile([P, SLOTS, HIDDEN], dt.float32, tag="G")
                nc.gpsimd.dma_gather(
                    out_ap=G[:, :, :],
                    in_ap=table_d[:, :],
                    idxs_ap=idx_sb[:, :],
                    num_idxs=P * SLOTS,
                    num_idxs_reg=P * SLOTS,
                    elem_size=HIDDEN,
                    single_packet=False,
                )

                # tree-reduce the 32 white slots and 32 black slots
                for base in (0, MAX_FEATS):
                    w = MAX_FEATS // 2
                    while w >= 1:
                        nc.vector.tensor_add(
                            G[:, base : base + w, :],
                            G[:, base : base + w, :],
                            G[:, base + w : base + 2 * w, :],
                        )
                        w //= 2

                wh = wpool.tile([P, HIDDEN], dt.float32, tag="wh")
                bh = wpool.tile([P, HIDDEN], dt.float32, tag="bh")
                nc.vector.tensor_add(wh[:], G[:, 0, :], bias_sb[:])
                nc.vector.tensor_add(bh[:], G[:, MAX_FEATS, :], bias_sb[:])
                # clip(x, 0, 127)
                nc.vector.tensor_scalar(
                    wh[:], wh[:], 0.0, 127.0, mybir.AluOpType.max, mybir.AluOpType.min
                )
                nc.vector.tensor_scalar(
                    bh[:], bh[:], 0.0, 127.0, mybir.AluOpType.max, mybir.AluOpType.min
                )

                # us = bh + s*(wh-bh); them = wh - s*(wh-bh)
                x = wpool.tile([P, 2 * HIDDEN], dt.float32, tag="x")
                diff = wpool.tile([P, HIDDEN], dt.float32, tag="diff")
                nc.vector.tensor_sub(diff[:], wh[:], bh[:])
                nc.vector.tensor_mul(
                    diff[:], diff[:], stm_sb[:].to_broadcast((P, HIDDEN))
                )
                nc.vector.tensor_add(x[:, 0:HIDDEN], bh[:], diff[:])
                nc.vector.tensor_sub(x[:, HIDDEN : 2 * HIDDEN], wh[:], diff[:])

                # transpose x -> xT chunks [128(k), 128(batch)]
                psum_x = ppool.tile([P, 4 * P], dt.float32, tag="px")
                for c in range(4):
                    nc.tensor.transpose(
                        out=psum_x[:, c * P : (c + 1) * P],
                        in_=x[:, c * P : (c + 1) * P],
                        identity=ident[:],
                    )
                xT = wpool.tile([P, 4, P], dt.float32, tag="xT")
                nc.vector.tensor_copy(xT[:], psum_x[:])

                # fc1: a1 [32, 128] = w1 @ x.T
                a1p = ppool.tile([32, P], dt.float32, tag="a1")
                for c in range(4):
                    nc.tensor.matmul(
                        out=a1p[:],
                        lhsT=w1t_sb[:, c, :],
                        rhs=xT[:, c, :],
                        start=(c == 0),
                        stop=(c == 3),
                    )
                a1 = wpool.tile([32, P], dt.float32, tag="a1s")
                nc.scalar.activation(
                    a1[:], a1p[:], mybir.ActivationFunctionType.Relu, bias=b1_sb[:, 0:1]
                )

                # fc2: a2 [32, 128]
                a2p = ppool.tile([32, P], dt.float32, tag="a2")
                nc.tensor.matmul(
                    out=a2p[:], lhsT=w2t_sb[:], rhs=a1[:], start=True, stop=True
                )
                a2 = wpool.tile([32, P], dt.float32, tag="a2s")
                nc.scalar.activation(
                    a2[:], a2p[:], mybir.ActivationFunctionType.Relu, bias=b2_sb[:, 0:1]
                )

                # out row: [1, 128], sigmoid(w_o @ a2 + bo)
                op = ppool.tile([1, P], dt.float32, tag="op")
                nc.tensor.matmul(
                    out=op[:], lhsT=wot_sb[:, 0:1], rhs=a2[:], start=True, stop=True
                )
                nc.scalar.activation(
                    out_sb[:, r0 : r0 + P],
                    op[:],
                    mybir.ActivationFunctionType.Sigmoid,
                    bias=bo_sb[:, 0:1],
                )

            nc.sync.dma_start(out_d[:, :], out_sb[:])

    nc.compile()
    return nc


_NC_CACHE = None


def _get_nc():
    global _NC_CACHE
    if _NC_CACHE is None:
        _NC_CACHE = build_nc()
    return _NC_CACHE


def prepare_maps(inputs):
    wf = np.asarray(inputs["white_features"]).astype(np.int64)
    bf = np.asarray(inputs["black_features"]).astype(np.int64)
    stm = np.asarray(inputs["stm"], dtype=np.float32).reshape(BATCH, 1)
    ft_W = np.asarray(inputs["ft_W"], dtype=np.float32)
    ft_b = np.asarray(inputs["ft_b"], dtype=np.float32).reshape(1, HIDDEN)
    w1 = np.asarray(inputs["w1"], dtype=np.float32)
    b1 = np.asarray(inputs["b1"], dtype=np.float32).reshape(32, 1)
    w2 = np.asarray(inputs["w2"], dtype=np.float32)
    b2 = np.asarray(inputs["b2"], dtype=np.float32).reshape(32, 1)
    wo = np.asarray(inputs["wo"], dtype=np.float32)
    bo = np.asarray(inputs["bo"], dtype=np.float32).reshape(1, 1)

    ft_Wt = np.ascontiguousarray(ft_W.T)  # [40960, 256]
    idx_all = np.concatenate([wf, bf], axis=1)  # [8192, 64], -1 padded

    w1t = np.ascontiguousarray(w1.T)  # [512, 32]
    w2t = np.ascontiguousarray(w2.T)  # [32, 32]
    wot = np.ascontiguousarray(wo.T)  # [32, 1]

    in_maps = []
    for c in range(N_CORES):
        rows = slice(c * ROWS, (c + 1) * ROWS)
        cidx = idx_all[rows]  # [1024, 64]
        valid = cidx >= 0
        uniq = np.unique(cidx[valid])  # sorted unique rows this core touches
        n_uniq = uniq.shape[0]
        assert n_uniq <= ZERO_ROW, f"core {c}: {n_uniq} unique rows > {ZERO_ROW}"
        table_c = np.zeros((TABLE_ROWS, HIDDEN), dtype=np.float32)
        table_c[:n_uniq] = ft_Wt[uniq]
        ids = np.searchsorted(uniq, np.where(valid, cidx, uniq[0])).astype(np.int16)
        ids = np.where(valid, ids, np.int16(ZERO_ROW))  # [1024, 64]

        # dma_gather stream: position s*128 + r carries slot s of row r, so
        # gathered row lands at G[r, s, :]. Wrap: index j -> [j%16, j//16],
        # replicated across the 8 partition groups of 16.
        ids_t = ids.reshape(N_TILES, P, SLOTS)  # [8, 128, 64]
        stream = ids_t.transpose(0, 2, 1).reshape(N_TILES, P * SLOTS)  # [8, 8192]
        wrap = stream.reshape(N_TILES, IDX_COLS, 16).transpose(0, 2, 1)  # [8,16,512]
        idx16 = np.ascontiguousarray(
            np.tile(wrap, (1, P // 16, 1))
        )  # [8, 128, 512] int16

        in_maps.append(
            {
                "idx": idx16,
                "table": table_c,
                "stm": stm[rows],
                "ftb": ft_b,
                "w1t": w1t,
                "b1": b1,
                "w2t": w2t,
                "b2": b2,
                "wot": wot,
                "bo": bo,
            }
        )
    return in_maps


def run(inputs, trace=False, trace_kwargs=None):
    from concourse.bass_utils import run_bass_kernel_spmd

    nc = _get_nc()
    in_maps = prepare_maps(inputs)
    res = run_bass_kernel_spmd(
        nc,
        in_maps,
        list(range(N_CORES)),
        trace=trace,
        **(trace_kwargs or {}),
    )
    out = np.concatenate(
        [res.results[c]["out"].reshape(-1) for c in range(N_CORES)]
    ).astype(np.float32)
    return out, res


def kernel(**inputs) -> np.ndarray:
    out, _ = run(inputs)
    return out
